# revision 35
# baseline (speedup 1.0000x reference)
"""Trainium2 Bass kernel for the region-attention module.

Computation (per batch b):
    attn1 = image[b] @ We.T + be            # [R, A]
    attn2 = dh[b] @ Wd.T + bd               # [A]
    scores = gelu(attn1) @ Wf1 + gelu(attn2) @ Wf2 + bf   # [R]
    alpha = softmax(scores)                 # [R]
    attended = alpha @ image[b]             # [C]

Sharding: data-parallel over batch across 8 NeuronCores (16 batches/core),
weights replicated.  No cross-device communication.

Per-core kernel strategy (B_CORE=16 batches, processed in 8 blocks of 2):
  - image rows (b,r flattened) loaded once in fp32 (contiguous DMA), kept in
    SBUF for stage 3.
  - stage-1 matmul needs C on partitions; DMA-transpose only supports 2-byte
    dtypes, so a host-precast bf16 copy of the image (an extra kernel input,
    padded to a multiple of 16 rows) is DMA-transpose-loaded as [c, rows].
    bf16 stage-1 keeps end-to-end relative error ~1e-3 (fp32 accumulate in
    PSUM).
  - float32r (fp32 with 11-bit mantissa, full-rate on the PE) is used for the
    small score/attended matmuls; walrus requires every producer of an f32r
    matmul operand to be f32r-typed, so those tiles/DRAM tensors are declared
    float32r (same byte width/bits as fp32).
  - attn1T[a, rows] = WeT_chunk.T @ imT_chunk accumulated over 16 c-chunks.
  - gelu+bias on ScalarE straight out of PSUM; scores via a skinny
    [128,1].T @ [128,rows] float32r matmul accumulated over 8 a-chunks.
  - softmax on [2, 196] tiles (VectorE/ScalarE).
  - stage 3: attended = alpha_blockdiag.T @ image_natural as float32r
    matmuls over row chunks (alpha scattered into a [128, 4, 2] block-diag
    operand by tiny SBUF->SBUF DMAs).
"""

import os
import sys

for _p in ("/opt/trn_rl_repo", "/root/.axon_site/_ro/trn_rl_repo"):
    if os.path.isdir(_p) and _p not in sys.path:
        sys.path.append(_p)

from contextlib import ExitStack

import ml_dtypes
import numpy as np

import concourse.bass as bass
import concourse.tile as tile
from concourse import bacc, mybir
from concourse.bass_utils import run_bass_kernel_spmd

AFT = mybir.ActivationFunctionType
DT = mybir.dt

# Problem shapes (hardcoded per contest contract)
B, R, C = 128, 196, 2048
H, A = 1024, 1024
N_CORES = 8
B_CORE = B // N_CORES        # 16 batches per core
NB = 2                       # batches per block
ROWS = NB * R                # 392 rows per block
ROWS_PAD = 400               # multiple of 16 for the xbar DMA transpose
N_BLOCKS = B_CORE // NB      # 8
N_ROWT = (ROWS + 127) // 128  # 4 row chunks per block (128,128,128,8)
CA = C // 128                # 16 c-chunks
AA = A // 128                # 8 a-chunks
HA = H // 128                # 8 h-chunks
CB = C // 512                # 4 column blocks for stage 3


def _row_cnt(t):
    return min(128, ROWS - t * 128)


def build_nc():
    f32, bf16, f32r = DT.float32, DT.bfloat16, DT.float32r
    nc = bacc.Bacc(
        "TRN2", target_bir_lowering=False, debug=False, num_devices=N_CORES
    )

    img = nc.dram_tensor("img", [B_CORE * R, C], f32r, kind="ExternalInput")
    imgb = nc.dram_tensor(
        "imgb", [N_BLOCKS * ROWS_PAD, C], bf16, kind="ExternalInput"
    )
    dht = nc.dram_tensor("dht", [H, B_CORE], bf16, kind="ExternalInput")
    wet = nc.dram_tensor("wet", [C, A], bf16, kind="ExternalInput")
    wdt = nc.dram_tensor("wdt", [H, A], bf16, kind="ExternalInput")
    be8 = nc.dram_tensor("be8", [128, AA], f32, kind="ExternalInput")
    bd8 = nc.dram_tensor("bd8", [128, AA], f32, kind="ExternalInput")
    wf1 = nc.dram_tensor("wf1", [128, AA], f32r, kind="ExternalInput")
    wf2 = nc.dram_tensor("wf2", [128, AA], f32r, kind="ExternalInput")
    bf1 = nc.dram_tensor("bf1", [1, 1], f32, kind="ExternalInput")

    att_out = nc.dram_tensor("att_out", [B_CORE, C], f32, kind="ExternalOutput")
    alpha_out = nc.dram_tensor("alpha_out", [B_CORE, R], f32, kind="ExternalOutput")

    with tile.TileContext(nc) as tc, ExitStack() as ctx:
        wet_pool = ctx.enter_context(tc.tile_pool(name="wet", bufs=1))
        const_pool = ctx.enter_context(tc.tile_pool(name="const", bufs=1))
        nat_pool = ctx.enter_context(tc.tile_pool(name="nat", bufs=6))
        imt_pool = ctx.enter_context(tc.tile_pool(name="imt", bufs=5))
        g1_pool = ctx.enter_context(tc.tile_pool(name="g1", bufs=8))
        sm_pool = ctx.enter_context(tc.tile_pool(name="sm", bufs=2))
        out_pool = ctx.enter_context(tc.tile_pool(name="outs", bufs=1))
        apsum = ctx.enter_context(tc.tile_pool(name="apsum", bufs=4, space="PSUM"))
        spsum = ctx.enter_context(tc.tile_pool(name="spsum", bufs=2, space="PSUM"))
        opsum = ctx.enter_context(tc.tile_pool(name="opsum", bufs=2, space="PSUM"))

        # ---- persistent constants
        be_sb = const_pool.tile([128, AA], f32, tag="be")
        nc.scalar.dma_start(be_sb[:], be8[:])
        bd_sb = const_pool.tile([128, AA], f32, tag="bd")
        nc.scalar.dma_start(bd_sb[:], bd8[:])
        # f32r-typed DMAs must use the SWDGE (gpsimd) path: HWDGE f32r
        # descriptors corrupt concurrent DMA-transpose traffic (HW bug found
        # empirically; see probe10 — sync f32r load scrambles xbar output).
        wf1_sb = const_pool.tile([128, AA], f32r, tag="wf1")
        nc.gpsimd.dma_start(wf1_sb[:], wf1[:])
        wf2_sb = const_pool.tile([128, AA], f32r, tag="wf2")
        nc.gpsimd.dma_start(wf2_sb[:], wf2[:])
        bf_sb = const_pool.tile([1, 1], f32, tag="bf")
        nc.scalar.dma_start(bf_sb[:], bf1[:])
        sc2_sb = const_pool.tile([1, B_CORE], f32, tag="sc2sb")
        # zero source for DMA-laundered f32r zero fills (memset can't target
        # f32r, and engine writers of f32r matmul operands must round)
        zrow = const_pool.tile([128, N_ROWT * NB], f32r, tag="zrow")
        nc.vector.memset(zrow[:].bitcast(f32), 0.0)

        # ---- phase 0: decoder branch -> sc2_sb[1, B_CORE]
        with (
            tc.tile_pool(name="ph0", bufs=1) as ph0,
            tc.tile_pool(name="ph0g", bufs=4) as ph0g,
        ):
            wdt_sb = []
            for h in range(HA):
                t = ph0.tile([128, A], bf16, tag=f"wdt{h}")
                nc.scalar.dma_start(t[:], wdt[h * 128 : (h + 1) * 128, :])
                wdt_sb.append(t)
            dht_sb = []
            for h in range(HA):
                t = ph0.tile([128, B_CORE], bf16, tag=f"dht{h}")
                nc.scalar.dma_start(t[:], dht[h * 128 : (h + 1) * 128, :])
                dht_sb.append(t)

            sc2_ps = spsum.tile([1, B_CORE], f32, tag="sc")
            for a in range(AA):
                a2_ps = apsum.tile([128, B_CORE], f32, tag="ap")
                for h in range(HA):
                    nc.tensor.matmul(
                        a2_ps[:],
                        wdt_sb[h][:, a * 128 : (a + 1) * 128],
                        dht_sb[h][:],
                        start=(h == 0),
                        stop=(h == HA - 1),
                    )
                g2 = ph0g.tile([128, B_CORE], f32r, tag="g2")
                nc.scalar.activation(
                    g2[:], a2_ps[:], AFT.Gelu, bias=bd_sb[:, a : a + 1], scale=1.0
                )
                nc.tensor.matmul(
                    sc2_ps[:],
                    wf2_sb[:, a : a + 1],
                    g2[:],
                    start=(a == 0),
                    stop=(a == AA - 1),
                )
            nc.vector.tensor_scalar_add(sc2_sb[:], sc2_ps[:], bf_sb[0:1, 0:1])

        # encoder weights (loaded while phase 0 computes on the PE)
        wet_sb = []
        for c in range(CA):
            t = wet_pool.tile([128, A], bf16, tag=f"wet{c}")
            nc.gpsimd.dma_start(t[:], wet[c * 128 : (c + 1) * 128, :])
            wet_sb.append(t)

        # ---- main loop over blocks of NB batches
        for nb in range(N_BLOCKS):
            r0 = nb * ROWS

            # natural fp32 loads (kept for stage 3)
            nat = []
            for t in range(N_ROWT):
                rt = _row_cnt(t)
                ntile = nat_pool.tile([128, C], f32r, tag="nat")
                nc.gpsimd.dma_start(
                    ntile[0:rt, :], img[r0 + t * 128 : r0 + t * 128 + rt, :]
                )
                nat.append(ntile)

            # transposed bf16 load straight from the host-precast copy, ONE
            # 3D-output xbar transpose per block: imt_all[p, c, r] =
            # imgb[r0+r, c*128+p].  Contiguous source rows -> few descriptors
            # (the per-c-chunk variant costs ~1.2us of HWDGE issue each).
            imt_all = imt_pool.tile([128, CA, ROWS_PAD], bf16, tag="imt")
            nc.sync.dma_start(
                imt_all[:], imgb[r0 : r0 + ROWS_PAD, :], transpose=True
            )

            # stage 1: attn1T chunks + gelu + score accumulation
            sc_ps = spsum.tile([1, ROWS], f32, tag="sc")
            for a in range(AA):
                ap_t = apsum.tile([128, ROWS], f32, tag="ap")
                for c in range(CA):
                    nc.tensor.matmul(
                        ap_t[:],
                        wet_sb[c][:, a * 128 : (a + 1) * 128],
                        imt_all[:, c, 0:ROWS],
                        start=(c == 0),
                        stop=(c == CA - 1),
                    )
                g1 = g1_pool.tile([128, ROWS], f32r, tag="g1")
                nc.scalar.activation(
                    g1[:], ap_t[:], AFT.Gelu, bias=be_sb[:, a : a + 1], scale=1.0
                )
                nc.tensor.matmul(
                    sc_ps[:],
                    wf1_sb[:, a : a + 1],
                    g1[:],
                    start=(a == 0),
                    stop=(a == AA - 1),
                )

            # softmax over regions, per batch
            sc_sb = sm_pool.tile([1, ROWS], f32, tag="scsb")
            nc.vector.tensor_copy(sc_sb[:], sc_ps[:])
            for b in range(NB):
                gb = nb * NB + b
                nc.vector.tensor_scalar_add(
                    sc_sb[0:1, b * R : (b + 1) * R],
                    sc_sb[0:1, b * R : (b + 1) * R],
                    sc2_sb[0:1, gb : gb + 1],
                )
            sc2d = sm_pool.tile([NB, R], f32, tag="sc2d")
            nc.gpsimd.dma_start(sc2d[:], sc_sb[:])
            negmx = sm_pool.tile([NB, 1], f32, tag="negmx")
            nc.vector.reduce_max(
                negmx[:], sc2d[:], axis=mybir.AxisListType.X, negate=True
            )
            # exp via tanh so ScalarE stays on the gelu table set (a gelu<->exp
            # switch costs a 1.3us ACT_TABLE_LOAD twice per block):
            # exp(x) = (1+t)/(1-t), t = tanh(x/2), x = scores - max <= 0
            nmh = sm_pool.tile([NB, 1], f32, tag="nmh")
            nc.vector.tensor_scalar_mul(nmh[:], negmx[:], 0.5)
            tnh = sm_pool.tile([NB, R], f32, tag="tnh")
            nc.scalar.activation(tnh[:], sc2d[:], AFT.Tanh, bias=nmh[:], scale=0.5)
            den = sm_pool.tile([NB, R], f32, tag="den")
            nc.vector.tensor_scalar(
                den[:], tnh[:], -1.0, 1.0,
                op0=mybir.AluOpType.mult, op1=mybir.AluOpType.add,
            )
            rcpd = sm_pool.tile([NB, R], f32, tag="rcpd")
            nc.vector.reciprocal(rcpd[:], den[:])
            num = sm_pool.tile([NB, R], f32, tag="num")
            nc.vector.tensor_scalar_add(num[:], tnh[:], 1.0)
            esb = sm_pool.tile([NB, R], f32, tag="esb")
            nc.vector.tensor_mul(esb[:], num[:], rcpd[:])
            ssum = sm_pool.tile([NB, 1], f32, tag="ssum")
            nc.vector.reduce_sum(ssum[:], esb[:], axis=mybir.AxisListType.X)
            rcp = sm_pool.tile([NB, 1], f32, tag="rcp")
            nc.vector.reciprocal(rcp[:], ssum[:])
            alpha_sb = sm_pool.tile([NB, R], f32r, tag="alpha")
            nc.vector.tensor_scalar_mul(alpha_sb[:], esb[:], rcp[:])
            nc.scalar.dma_start(
                alpha_out[nb * NB : (nb + 1) * NB, :],
                alpha_sb[:].bitcast(f32),
            )

            # scatter alpha into block-diagonal stationary operand
            # bd_t[p, t, b] = alpha of block-row (t*128+p) if that row belongs
            # to batch b (rows [b*R, (b+1)*R) within the block), else 0.
            bd_t = sm_pool.tile([128, N_ROWT, NB], f32r, tag="bdt")
            nc.gpsimd.dma_start(bd_t[:], zrow[:])
            for b in range(NB):
                for t in range(N_ROWT):
                    rs = max(b * R, t * 128)
                    re_ = min((b + 1) * R, (t + 1) * 128)
                    if rs >= re_:
                        continue
                    nc.gpsimd.dma_start(
                        bd_t[rs - t * 128 : re_ - t * 128, t, b],
                        alpha_sb[b : b + 1, rs - b * R : re_ - b * R],
                    )

            # stage 3: attended = alpha_blockdiag.T @ image_natural
            att_sb = out_pool.tile([NB, C], f32, tag="attsb")
            for cb in range(CB):
                at_ps = opsum.tile([NB, 512], f32, tag="atps")
                for t in range(N_ROWT):
                    rt = _row_cnt(t)
                    nc.tensor.matmul(
                        at_ps[:],
                        bd_t[0:rt, t, :],
                        nat[t][0:rt, cb * 512 : (cb + 1) * 512],
                        start=(t == 0),
                        stop=(t == N_ROWT - 1),
                    )
                nc.vector.tensor_copy(att_sb[:, cb * 512 : (cb + 1) * 512], at_ps[:])
            nc.scalar.dma_start(att_out[nb * NB : (nb + 1) * NB, :], att_sb[:])

    nc.compile()
    return nc


def make_in_maps(inputs):
    """Split full inputs into per-core input maps (host-side prep)."""
    image = np.asarray(inputs["image"], dtype=np.float32)
    dh = np.asarray(inputs["decoder_hidden"], dtype=np.float32)
    We = np.asarray(inputs["We"], dtype=np.float32)
    be = np.asarray(inputs["be"], dtype=np.float32)
    Wd = np.asarray(inputs["Wd"], dtype=np.float32)
    bd = np.asarray(inputs["bd"], dtype=np.float32)
    Wf = np.asarray(inputs["Wf"], dtype=np.float32)
    bf = np.asarray(inputs["bf"], dtype=np.float32)

    bf16 = ml_dtypes.bfloat16
    wet = np.ascontiguousarray(We.T).astype(bf16)              # [C, A]
    wdt = np.ascontiguousarray(Wd.T).astype(bf16)              # [H, A]
    be8 = np.ascontiguousarray(be.reshape(AA, 128).T)          # [128, AA]
    bd8 = np.ascontiguousarray(bd.reshape(AA, 128).T)
    wf1 = np.ascontiguousarray(Wf[0, :A].reshape(AA, 128).T)
    wf2 = np.ascontiguousarray(Wf[0, A:].reshape(AA, 128).T)
    bf1 = bf.reshape(1, 1)

    image_bf = image.astype(bf16)  # host precast for the transposed loads

    in_maps = []
    for i in range(N_CORES):
        sl = slice(i * B_CORE, (i + 1) * B_CORE)
        imgb = np.zeros((N_BLOCKS * ROWS_PAD, C), dtype=bf16)
        imgb[: B_CORE * R] = image_bf[sl].reshape(B_CORE * R, C)
        in_maps.append(
            {
                "img": np.ascontiguousarray(
                    image[sl].reshape(B_CORE * R, C)
                ),
                "imgb": imgb,
                "dht": np.ascontiguousarray(dh[sl].T).astype(bf16),
                "wet": wet,
                "wdt": wdt,
                "be8": be8,
                "bd8": bd8,
                "wf1": wf1,
                "wf2": wf2,
                "bf1": bf1,
            }
        )
    return in_maps


_NC_CACHE = {}


def kernel(**inputs):
    if "nc" not in _NC_CACHE:
        _NC_CACHE["nc"] = build_nc()
    nc = _NC_CACHE["nc"]
    in_maps = make_in_maps(inputs)
    res = run_bass_kernel_spmd(nc, in_maps, core_ids=list(range(N_CORES)))
    kernel.last_result = res
    attended = np.concatenate([res.results[i]["att_out"] for i in range(N_CORES)], 0)
    alpha = np.concatenate([res.results[i]["alpha_out"] for i in range(N_CORES)], 0)
    return attended, alpha


kernel.last_result = None


# revision 38
# speedup vs baseline: 1.0240x; 1.0240x over previous
"""Trainium2 Bass kernel for the region-attention module.

Computation (per batch b):
    attn1 = image[b] @ We.T + be            # [R, A]
    attn2 = dh[b] @ Wd.T + bd               # [A]
    scores = gelu(attn1) @ Wf1 + gelu(attn2) @ Wf2 + bf   # [R]
    alpha = softmax(scores)                 # [R]
    attended = alpha @ image[b]             # [C]

Sharding: data-parallel over batch across 8 NeuronCores (16 batches/core),
weights replicated.  No cross-device communication.

Per-core kernel strategy (B_CORE=16 batches, processed in 8 blocks of 2):
  - image rows (b,r flattened) loaded once in fp32 (contiguous DMA), kept in
    SBUF for stage 3.
  - stage-1 matmul needs C on partitions; DMA-transpose only supports 2-byte
    dtypes, so a host-precast bf16 copy of the image (an extra kernel input,
    padded to a multiple of 16 rows) is DMA-transpose-loaded as [c, rows].
    bf16 stage-1 keeps end-to-end relative error ~1e-3 (fp32 accumulate in
    PSUM).
  - float32r (fp32 with 11-bit mantissa, full-rate on the PE) is used for the
    small score/attended matmuls; walrus requires every producer of an f32r
    matmul operand to be f32r-typed, so those tiles/DRAM tensors are declared
    float32r (same byte width/bits as fp32).
  - attn1T[a, rows] = WeT_chunk.T @ imT_chunk accumulated over 16 c-chunks.
  - gelu+bias on ScalarE straight out of PSUM; scores via a skinny
    [128,1].T @ [128,rows] float32r matmul accumulated over 8 a-chunks.
  - softmax on [2, 196] tiles (VectorE/ScalarE).
  - stage 3: attended = alpha_blockdiag.T @ image_natural as float32r
    matmuls over row chunks (alpha scattered into a [128, 4, 2] block-diag
    operand by tiny SBUF->SBUF DMAs).
"""

import os
import sys

for _p in ("/opt/trn_rl_repo", "/root/.axon_site/_ro/trn_rl_repo"):
    if os.path.isdir(_p) and _p not in sys.path:
        sys.path.append(_p)

from contextlib import ExitStack

import ml_dtypes
import numpy as np

import concourse.bass as bass
import concourse.tile as tile
from concourse import bacc, mybir
from concourse.bass_utils import run_bass_kernel_spmd

AFT = mybir.ActivationFunctionType
DT = mybir.dt

# Problem shapes (hardcoded per contest contract)
B, R, C = 128, 196, 2048
H, A = 1024, 1024
N_CORES = 8
B_CORE = B // N_CORES        # 16 batches per core
NB = 2                       # batches per block
ROWS = NB * R                # 392 rows per block
ROWS_PAD = 400               # multiple of 16 for the xbar DMA transpose
N_BLOCKS = B_CORE // NB      # 8
N_ROWT = (ROWS + 127) // 128  # 4 row chunks per block (128,128,128,8)
CA = C // 128                # 16 c-chunks
AA = A // 128                # 8 a-chunks
HA = H // 128                # 8 h-chunks
CB = C // 512                # 4 column blocks for stage 3


def _row_cnt(t):
    return min(128, ROWS - t * 128)


def build_nc():
    f32, bf16, f32r = DT.float32, DT.bfloat16, DT.float32r
    nc = bacc.Bacc(
        "TRN2", target_bir_lowering=False, debug=False, num_devices=N_CORES
    )

    img = nc.dram_tensor("img", [B_CORE * R, C], f32r, kind="ExternalInput")
    imgb = nc.dram_tensor(
        "imgb", [N_BLOCKS * ROWS_PAD, C], bf16, kind="ExternalInput"
    )
    dht = nc.dram_tensor("dht", [H, B_CORE], bf16, kind="ExternalInput")
    wet = nc.dram_tensor("wet", [C, A], bf16, kind="ExternalInput")
    wdt = nc.dram_tensor("wdt", [H, A], bf16, kind="ExternalInput")
    be8 = nc.dram_tensor("be8", [128, AA], f32, kind="ExternalInput")
    bd8 = nc.dram_tensor("bd8", [128, AA], f32, kind="ExternalInput")
    wf1 = nc.dram_tensor("wf1", [128, AA], f32r, kind="ExternalInput")
    wf2 = nc.dram_tensor("wf2", [128, AA], f32r, kind="ExternalInput")
    bf1 = nc.dram_tensor("bf1", [1, 1], f32, kind="ExternalInput")

    att_out = nc.dram_tensor("att_out", [B_CORE, C], f32, kind="ExternalOutput")
    alpha_out = nc.dram_tensor("alpha_out", [B_CORE, R], f32, kind="ExternalOutput")

    with tile.TileContext(nc) as tc, ExitStack() as ctx:
        wet_pool = ctx.enter_context(tc.tile_pool(name="wet", bufs=1))
        const_pool = ctx.enter_context(tc.tile_pool(name="const", bufs=1))
        nat_pool = ctx.enter_context(tc.tile_pool(name="nat", bufs=6))
        imt_pool = ctx.enter_context(tc.tile_pool(name="imt", bufs=5))
        g1_pool = ctx.enter_context(tc.tile_pool(name="g1", bufs=8))
        sm_pool = ctx.enter_context(tc.tile_pool(name="sm", bufs=2))
        out_pool = ctx.enter_context(tc.tile_pool(name="outs", bufs=1))
        apsum = ctx.enter_context(tc.tile_pool(name="apsum", bufs=4, space="PSUM"))
        spsum = ctx.enter_context(tc.tile_pool(name="spsum", bufs=2, space="PSUM"))
        opsum = ctx.enter_context(tc.tile_pool(name="opsum", bufs=2, space="PSUM"))

        # ---- persistent constants
        be_sb = const_pool.tile([128, AA], f32, tag="be")
        nc.scalar.dma_start(be_sb[:], be8[:])
        bd_sb = const_pool.tile([128, AA], f32, tag="bd")
        nc.scalar.dma_start(bd_sb[:], bd8[:])
        # f32r-typed DMAs must use the SWDGE (gpsimd) path: HWDGE f32r
        # descriptors corrupt concurrent DMA-transpose traffic (HW bug found
        # empirically; see probe10 — sync f32r load scrambles xbar output).
        wf1_sb = const_pool.tile([128, AA], f32r, tag="wf1")
        nc.gpsimd.dma_start(wf1_sb[:], wf1[:])
        wf2_sb = const_pool.tile([128, AA], f32r, tag="wf2")
        nc.gpsimd.dma_start(wf2_sb[:], wf2[:])
        bf_sb = const_pool.tile([1, 1], f32, tag="bf")
        nc.scalar.dma_start(bf_sb[:], bf1[:])
        sc2_sb = const_pool.tile([1, B_CORE], f32, tag="sc2sb")
        # zero source for DMA-laundered f32r zero fills (memset can't target
        # f32r, and engine writers of f32r matmul operands must round)
        zrow = const_pool.tile([128, N_ROWT * NB], f32r, tag="zrow")
        nc.vector.memset(zrow[:].bitcast(f32), 0.0)

        # ---- phase 0: decoder branch -> sc2_sb[1, B_CORE]
        with (
            tc.tile_pool(name="ph0", bufs=1) as ph0,
            tc.tile_pool(name="ph0g", bufs=4) as ph0g,
        ):
            wdt_sb = []
            for h in range(HA):
                t = ph0.tile([128, A], bf16, tag=f"wdt{h}")
                nc.scalar.dma_start(t[:], wdt[h * 128 : (h + 1) * 128, :])
                wdt_sb.append(t)
            dht_sb = []
            for h in range(HA):
                t = ph0.tile([128, B_CORE], bf16, tag=f"dht{h}")
                nc.scalar.dma_start(t[:], dht[h * 128 : (h + 1) * 128, :])
                dht_sb.append(t)

            sc2_ps = spsum.tile([1, B_CORE], f32, tag="sc")
            for a in range(AA):
                a2_ps = apsum.tile([128, B_CORE], f32, tag="ap")
                for h in range(HA):
                    nc.tensor.matmul(
                        a2_ps[:],
                        wdt_sb[h][:, a * 128 : (a + 1) * 128],
                        dht_sb[h][:],
                        start=(h == 0),
                        stop=(h == HA - 1),
                    )
                g2 = ph0g.tile([128, B_CORE], f32r, tag="g2")
                nc.scalar.activation(
                    g2[:], a2_ps[:], AFT.Gelu, bias=bd_sb[:, a : a + 1], scale=1.0
                )
                nc.tensor.matmul(
                    sc2_ps[:],
                    wf2_sb[:, a : a + 1],
                    g2[:],
                    start=(a == 0),
                    stop=(a == AA - 1),
                )
            nc.vector.tensor_scalar_add(sc2_sb[:], sc2_ps[:], bf_sb[0:1, 0:1])

        # encoder weights (loaded while phase 0 computes on the PE)
        wet_sb = []
        for c in range(CA):
            t = wet_pool.tile([128, A], bf16, tag=f"wet{c}")
            nc.gpsimd.dma_start(t[:], wet[c * 128 : (c + 1) * 128, :])
            wet_sb.append(t)

        def stage3(nb, bd_t):
            """attended(nb) = alpha_blockdiag.T @ image_natural; emitted one
            block late so a slow softmax never head-of-line-blocks the PE."""
            r0 = nb * ROWS
            nat = []
            for t in range(N_ROWT):
                rt = _row_cnt(t)
                ntile = nat_pool.tile([128, C], f32r, tag="nat")
                nc.gpsimd.dma_start(
                    ntile[0:rt, :], img[r0 + t * 128 : r0 + t * 128 + rt, :]
                )
                nat.append(ntile)
            att_sb = out_pool.tile([NB, C], f32, tag="attsb")
            for cb in range(CB):
                at_ps = opsum.tile([NB, 512], f32, tag="atps")
                for t in range(N_ROWT):
                    rt = _row_cnt(t)
                    nc.tensor.matmul(
                        at_ps[:],
                        bd_t[0:rt, t, :],
                        nat[t][0:rt, cb * 512 : (cb + 1) * 512],
                        start=(t == 0),
                        stop=(t == N_ROWT - 1),
                    )
                nc.vector.tensor_copy(att_sb[:, cb * 512 : (cb + 1) * 512], at_ps[:])
            nc.scalar.dma_start(att_out[nb * NB : (nb + 1) * NB, :], att_sb[:])

        # ---- main loop over blocks of NB batches
        prev_bd = None
        for nb in range(N_BLOCKS):
            r0 = nb * ROWS

            # transposed bf16 load straight from the host-precast copy, ONE
            # 3D-output xbar transpose per block: imt_all[p, c, r] =
            # imgb[r0+r, c*128+p].  Contiguous source rows -> few descriptors
            # (the per-c-chunk variant costs ~1.2us of HWDGE issue each).
            imt_all = imt_pool.tile([128, CA, ROWS_PAD], bf16, tag="imt")
            nc.sync.dma_start(
                imt_all[:], imgb[r0 : r0 + ROWS_PAD, :], transpose=True
            )

            # stage 1: attn1T chunks + gelu + score accumulation
            sc_ps = spsum.tile([1, ROWS], f32, tag="sc")
            for a in range(AA):
                ap_t = apsum.tile([128, ROWS], f32, tag="ap")
                for c in range(CA):
                    nc.tensor.matmul(
                        ap_t[:],
                        wet_sb[c][:, a * 128 : (a + 1) * 128],
                        imt_all[:, c, 0:ROWS],
                        start=(c == 0),
                        stop=(c == CA - 1),
                    )
                g1 = g1_pool.tile([128, ROWS], f32r, tag="g1")
                nc.scalar.activation(
                    g1[:], ap_t[:], AFT.Gelu, bias=be_sb[:, a : a + 1], scale=1.0
                )
                nc.tensor.matmul(
                    sc_ps[:],
                    wf1_sb[:, a : a + 1],
                    g1[:],
                    start=(a == 0),
                    stop=(a == AA - 1),
                )

            # softmax over regions, per batch
            sc_sb = sm_pool.tile([1, ROWS], f32, tag="scsb")
            nc.vector.tensor_copy(sc_sb[:], sc_ps[:])
            for b in range(NB):
                gb = nb * NB + b
                nc.vector.tensor_scalar_add(
                    sc_sb[0:1, b * R : (b + 1) * R],
                    sc_sb[0:1, b * R : (b + 1) * R],
                    sc2_sb[0:1, gb : gb + 1],
                )
            sc2d = sm_pool.tile([NB, R], f32, tag="sc2d")
            nc.scalar.dma_start(sc2d[:], sc_sb[:])
            negmx = sm_pool.tile([NB, 1], f32, tag="negmx")
            nc.vector.reduce_max(
                negmx[:], sc2d[:], axis=mybir.AxisListType.X, negate=True
            )
            # exp via tanh so ScalarE stays on the gelu table set (a gelu<->exp
            # switch costs a 1.3us ACT_TABLE_LOAD twice per block):
            # exp(x) = (1+t)/(1-t), t = tanh(x/2), x = scores - max <= 0
            nmh = sm_pool.tile([NB, 1], f32, tag="nmh")
            nc.vector.tensor_scalar_mul(nmh[:], negmx[:], 0.5)
            tnh = sm_pool.tile([NB, R], f32, tag="tnh")
            nc.scalar.activation(tnh[:], sc2d[:], AFT.Tanh, bias=nmh[:], scale=0.5)
            den = sm_pool.tile([NB, R], f32, tag="den")
            nc.vector.tensor_scalar(
                den[:], tnh[:], -1.0, 1.0,
                op0=mybir.AluOpType.mult, op1=mybir.AluOpType.add,
            )
            rcpd = sm_pool.tile([NB, R], f32, tag="rcpd")
            nc.vector.reciprocal(rcpd[:], den[:])
            num = sm_pool.tile([NB, R], f32, tag="num")
            nc.vector.tensor_scalar_add(num[:], tnh[:], 1.0)
            esb = sm_pool.tile([NB, R], f32, tag="esb")
            nc.vector.tensor_mul(esb[:], num[:], rcpd[:])
            ssum = sm_pool.tile([NB, 1], f32, tag="ssum")
            nc.vector.reduce_sum(ssum[:], esb[:], axis=mybir.AxisListType.X)
            rcp = sm_pool.tile([NB, 1], f32, tag="rcp")
            nc.vector.reciprocal(rcp[:], ssum[:])
            alpha_sb = sm_pool.tile([NB, R], f32r, tag="alpha")
            nc.vector.tensor_scalar_mul(alpha_sb[:], esb[:], rcp[:])
            nc.scalar.dma_start(
                alpha_out[nb * NB : (nb + 1) * NB, :],
                alpha_sb[:].bitcast(f32),
            )

            # scatter alpha into block-diagonal stationary operand
            # bd_t[p, t, b] = alpha of block-row (t*128+p) if that row belongs
            # to batch b (rows [b*R, (b+1)*R) within the block), else 0.
            bd_t = sm_pool.tile([128, N_ROWT, NB], f32r, tag="bdt")
            nc.gpsimd.dma_start(bd_t[:], zrow[:])
            for b in range(NB):
                for t in range(N_ROWT):
                    rs = max(b * R, t * 128)
                    re_ = min((b + 1) * R, (t + 1) * 128)
                    if rs >= re_:
                        continue
                    nc.gpsimd.dma_start(
                        bd_t[rs - t * 128 : re_ - t * 128, t, b],
                        alpha_sb[b : b + 1, rs - b * R : re_ - b * R],
                    )

            # deferred stage 3 of the previous block
            if prev_bd is not None:
                stage3(nb - 1, prev_bd)
            prev_bd = bd_t

        stage3(N_BLOCKS - 1, prev_bd)

    nc.compile()
    return nc


def make_in_maps(inputs):
    """Split full inputs into per-core input maps (host-side prep)."""
    image = np.asarray(inputs["image"], dtype=np.float32)
    dh = np.asarray(inputs["decoder_hidden"], dtype=np.float32)
    We = np.asarray(inputs["We"], dtype=np.float32)
    be = np.asarray(inputs["be"], dtype=np.float32)
    Wd = np.asarray(inputs["Wd"], dtype=np.float32)
    bd = np.asarray(inputs["bd"], dtype=np.float32)
    Wf = np.asarray(inputs["Wf"], dtype=np.float32)
    bf = np.asarray(inputs["bf"], dtype=np.float32)

    bf16 = ml_dtypes.bfloat16
    wet = np.ascontiguousarray(We.T).astype(bf16)              # [C, A]
    wdt = np.ascontiguousarray(Wd.T).astype(bf16)              # [H, A]
    be8 = np.ascontiguousarray(be.reshape(AA, 128).T)          # [128, AA]
    bd8 = np.ascontiguousarray(bd.reshape(AA, 128).T)
    wf1 = np.ascontiguousarray(Wf[0, :A].reshape(AA, 128).T)
    wf2 = np.ascontiguousarray(Wf[0, A:].reshape(AA, 128).T)
    bf1 = bf.reshape(1, 1)

    image_bf = image.astype(bf16)  # host precast for the transposed loads

    in_maps = []
    for i in range(N_CORES):
        sl = slice(i * B_CORE, (i + 1) * B_CORE)
        imgb = np.zeros((N_BLOCKS * ROWS_PAD, C), dtype=bf16)
        imgb[: B_CORE * R] = image_bf[sl].reshape(B_CORE * R, C)
        in_maps.append(
            {
                "img": np.ascontiguousarray(
                    image[sl].reshape(B_CORE * R, C)
                ),
                "imgb": imgb,
                "dht": np.ascontiguousarray(dh[sl].T).astype(bf16),
                "wet": wet,
                "wdt": wdt,
                "be8": be8,
                "bd8": bd8,
                "wf1": wf1,
                "wf2": wf2,
                "bf1": bf1,
            }
        )
    return in_maps


_NC_CACHE = {}


def kernel(**inputs):
    if "nc" not in _NC_CACHE:
        _NC_CACHE["nc"] = build_nc()
    nc = _NC_CACHE["nc"]
    in_maps = make_in_maps(inputs)
    res = run_bass_kernel_spmd(nc, in_maps, core_ids=list(range(N_CORES)))
    kernel.last_result = res
    attended = np.concatenate([res.results[i]["att_out"] for i in range(N_CORES)], 0)
    alpha = np.concatenate([res.results[i]["alpha_out"] for i in range(N_CORES)], 0)
    return attended, alpha


kernel.last_result = None


# revision 40
# speedup vs baseline: 1.2456x; 1.2164x over previous
"""Trainium2 Bass kernel for the region-attention module.

Computation (per batch b):
    attn1 = image[b] @ We.T + be            # [R, A]
    attn2 = dh[b] @ Wd.T + bd               # [A]
    scores = gelu(attn1) @ Wf1 + gelu(attn2) @ Wf2 + bf   # [R]
    alpha = softmax(scores)                 # [R]
    attended = alpha @ image[b]             # [C]

Sharding: data-parallel over batch across 8 NeuronCores (16 batches/core),
weights replicated.  No cross-device communication.

Per-core kernel strategy (B_CORE=16 batches, processed in 8 blocks of 2):
  - image rows (b,r flattened) loaded once in fp32 (contiguous DMA), kept in
    SBUF for stage 3.
  - stage-1 matmul needs C on partitions; a host-precast, host-transposed
    bf16 copy of the image (an extra kernel input [C, rows]) is loaded with
    one ordinary 3D-AP DMA per block (on-device DMA-transposes act as global
    DMA-serialization barriers and are bf16-only anyway).  bf16 stage-1
    keeps end-to-end relative error ~1e-3 (fp32 accumulate in PSUM).
  - float32r (fp32 with 11-bit mantissa, full-rate on the PE) is used for the
    small score/attended matmuls; walrus requires every producer of an f32r
    matmul operand to be f32r-typed, so those tiles/DRAM tensors are declared
    float32r (same byte width/bits as fp32).
  - attn1T[a, rows] = WeT_chunk.T @ imT_chunk accumulated over 16 c-chunks.
  - gelu+bias on ScalarE straight out of PSUM; scores via a skinny
    [128,1].T @ [128,rows] float32r matmul accumulated over 8 a-chunks.
  - softmax on [2, 196] tiles (VectorE/ScalarE).
  - stage 3: attended = alpha_blockdiag.T @ image_natural as float32r
    matmuls over row chunks (alpha scattered into a [128, 4, 2] block-diag
    operand by tiny SBUF->SBUF DMAs).
"""

import os
import sys

for _p in ("/opt/trn_rl_repo", "/root/.axon_site/_ro/trn_rl_repo"):
    if os.path.isdir(_p) and _p not in sys.path:
        sys.path.append(_p)

from contextlib import ExitStack

import ml_dtypes
import numpy as np

import concourse.bass as bass
import concourse.tile as tile
from concourse import bacc, mybir
from concourse.bass_utils import run_bass_kernel_spmd

AFT = mybir.ActivationFunctionType
DT = mybir.dt

# Problem shapes (hardcoded per contest contract)
B, R, C = 128, 196, 2048
H, A = 1024, 1024
N_CORES = 8
B_CORE = B // N_CORES        # 16 batches per core
NB = 2                       # batches per block
ROWS = NB * R                # 392 rows per block
PAD_ROWS = 3200              # padded total rows of the transposed bf16 image
N_BLOCKS = B_CORE // NB      # 8
N_ROWT = (ROWS + 127) // 128  # 4 row chunks per block (128,128,128,8)
CA = C // 128                # 16 c-chunks
AA = A // 128                # 8 a-chunks
HA = H // 128                # 8 h-chunks
CB = C // 512                # 4 column blocks for stage 3


def _row_cnt(t):
    return min(128, ROWS - t * 128)


def build_nc():
    f32, bf16, f32r = DT.float32, DT.bfloat16, DT.float32r
    nc = bacc.Bacc(
        "TRN2", target_bir_lowering=False, debug=False, num_devices=N_CORES
    )

    img = nc.dram_tensor("img", [B_CORE * R, C], f32r, kind="ExternalInput")
    imgbt = nc.dram_tensor("imgbt", [C, PAD_ROWS], bf16, kind="ExternalInput")
    dht = nc.dram_tensor("dht", [H, B_CORE], bf16, kind="ExternalInput")
    wet = nc.dram_tensor("wet", [C, A], bf16, kind="ExternalInput")
    wdt = nc.dram_tensor("wdt", [H, A], bf16, kind="ExternalInput")
    be8 = nc.dram_tensor("be8", [128, AA], f32, kind="ExternalInput")
    bd8 = nc.dram_tensor("bd8", [128, AA], f32, kind="ExternalInput")
    wf1 = nc.dram_tensor("wf1", [128, AA], f32r, kind="ExternalInput")
    wf2 = nc.dram_tensor("wf2", [128, AA], f32r, kind="ExternalInput")
    bf1 = nc.dram_tensor("bf1", [1, 1], f32, kind="ExternalInput")

    att_out = nc.dram_tensor("att_out", [B_CORE, C], f32, kind="ExternalOutput")
    alpha_out = nc.dram_tensor("alpha_out", [B_CORE, R], f32, kind="ExternalOutput")

    with tile.TileContext(nc) as tc, ExitStack() as ctx:
        wet_pool = ctx.enter_context(tc.tile_pool(name="wet", bufs=1))
        const_pool = ctx.enter_context(tc.tile_pool(name="const", bufs=1))
        nat_pool = ctx.enter_context(tc.tile_pool(name="nat", bufs=6))
        imt_pool = ctx.enter_context(tc.tile_pool(name="imt", bufs=5))
        g1_pool = ctx.enter_context(tc.tile_pool(name="g1", bufs=8))
        sm_pool = ctx.enter_context(tc.tile_pool(name="sm", bufs=2))
        out_pool = ctx.enter_context(tc.tile_pool(name="outs", bufs=1))
        apsum = ctx.enter_context(tc.tile_pool(name="apsum", bufs=4, space="PSUM"))
        spsum = ctx.enter_context(tc.tile_pool(name="spsum", bufs=2, space="PSUM"))
        opsum = ctx.enter_context(tc.tile_pool(name="opsum", bufs=2, space="PSUM"))

        # ---- persistent constants
        be_sb = const_pool.tile([128, AA], f32, tag="be")
        nc.scalar.dma_start(be_sb[:], be8[:])
        bd_sb = const_pool.tile([128, AA], f32, tag="bd")
        nc.scalar.dma_start(bd_sb[:], bd8[:])
        # f32r-typed DMAs must use the SWDGE (gpsimd) path: HWDGE f32r
        # descriptors corrupt concurrent DMA-transpose traffic (HW bug found
        # empirically; see probe10 — sync f32r load scrambles xbar output).
        wf1_sb = const_pool.tile([128, AA], f32r, tag="wf1")
        nc.gpsimd.dma_start(wf1_sb[:], wf1[:])
        wf2_sb = const_pool.tile([128, AA], f32r, tag="wf2")
        nc.gpsimd.dma_start(wf2_sb[:], wf2[:])
        bf_sb = const_pool.tile([1, 1], f32, tag="bf")
        nc.scalar.dma_start(bf_sb[:], bf1[:])
        sc2_sb = const_pool.tile([1, B_CORE], f32, tag="sc2sb")
        # zero source for DMA-laundered f32r zero fills (memset can't target
        # f32r, and engine writers of f32r matmul operands must round)
        zrow = const_pool.tile([128, N_ROWT * NB], f32r, tag="zrow")
        nc.vector.memset(zrow[:].bitcast(f32), 0.0)

        # ---- phase 0: decoder branch -> sc2_sb[1, B_CORE]
        with (
            tc.tile_pool(name="ph0", bufs=1) as ph0,
            tc.tile_pool(name="ph0g", bufs=4) as ph0g,
        ):
            wdt_sb = []
            for h in range(HA):
                t = ph0.tile([128, A], bf16, tag=f"wdt{h}")
                nc.scalar.dma_start(t[:], wdt[h * 128 : (h + 1) * 128, :])
                wdt_sb.append(t)
            dht_sb = []
            for h in range(HA):
                t = ph0.tile([128, B_CORE], bf16, tag=f"dht{h}")
                nc.scalar.dma_start(t[:], dht[h * 128 : (h + 1) * 128, :])
                dht_sb.append(t)

            sc2_ps = spsum.tile([1, B_CORE], f32, tag="sc")
            for a in range(AA):
                a2_ps = apsum.tile([128, B_CORE], f32, tag="ap")
                for h in range(HA):
                    nc.tensor.matmul(
                        a2_ps[:],
                        wdt_sb[h][:, a * 128 : (a + 1) * 128],
                        dht_sb[h][:],
                        start=(h == 0),
                        stop=(h == HA - 1),
                    )
                g2 = ph0g.tile([128, B_CORE], f32r, tag="g2")
                nc.scalar.activation(
                    g2[:], a2_ps[:], AFT.Gelu, bias=bd_sb[:, a : a + 1], scale=1.0
                )
                nc.tensor.matmul(
                    sc2_ps[:],
                    wf2_sb[:, a : a + 1],
                    g2[:],
                    start=(a == 0),
                    stop=(a == AA - 1),
                )
            nc.vector.tensor_scalar_add(sc2_sb[:], sc2_ps[:], bf_sb[0:1, 0:1])

        # encoder weights (loaded while phase 0 computes on the PE)
        wet_sb = []
        for c in range(CA):
            t = wet_pool.tile([128, A], bf16, tag=f"wet{c}")
            nc.gpsimd.dma_start(t[:], wet[c * 128 : (c + 1) * 128, :])
            wet_sb.append(t)

        def stage3(nb, bd_t):
            """attended(nb) = alpha_blockdiag.T @ image_natural; emitted one
            block late so a slow softmax never head-of-line-blocks the PE."""
            r0 = nb * ROWS
            nat = []
            for t in range(N_ROWT):
                rt = _row_cnt(t)
                ntile = nat_pool.tile([128, C], f32r, tag="nat")
                nc.gpsimd.dma_start(
                    ntile[0:rt, :], img[r0 + t * 128 : r0 + t * 128 + rt, :]
                )
                nat.append(ntile)
            att_sb = out_pool.tile([NB, C], f32, tag="attsb")
            for cb in range(CB):
                at_ps = opsum.tile([NB, 512], f32, tag="atps")
                for t in range(N_ROWT):
                    rt = _row_cnt(t)
                    nc.tensor.matmul(
                        at_ps[:],
                        bd_t[0:rt, t, :],
                        nat[t][0:rt, cb * 512 : (cb + 1) * 512],
                        start=(t == 0),
                        stop=(t == N_ROWT - 1),
                    )
                nc.vector.tensor_copy(att_sb[:, cb * 512 : (cb + 1) * 512], at_ps[:])
            nc.scalar.dma_start(att_out[nb * NB : (nb + 1) * NB, :], att_sb[:])

        # ---- main loop over blocks of NB batches
        prev_bd = None
        for nb in range(N_BLOCKS):
            r0 = nb * ROWS

            # transposed-layout bf16 load from the host-transposed copy:
            # imt_all[p, c, r] = imgbt[c*128+p, r0+r], one 3D-AP DMA
            imt_all = imt_pool.tile([128, CA, ROWS], bf16, tag="imt")
            nc.sync.dma_start(
                imt_all[:],
                imgbt[:, r0 : r0 + ROWS].rearrange("(c p) r -> p c r", p=128),
            )

            # stage 1: attn1T chunks + gelu + score accumulation
            sc_ps = spsum.tile([1, ROWS], f32, tag="sc")
            for a in range(AA):
                ap_t = apsum.tile([128, ROWS], f32, tag="ap")
                for c in range(CA):
                    nc.tensor.matmul(
                        ap_t[:],
                        wet_sb[c][:, a * 128 : (a + 1) * 128],
                        imt_all[:, c, 0:ROWS],
                        start=(c == 0),
                        stop=(c == CA - 1),
                    )
                g1 = g1_pool.tile([128, ROWS], f32r, tag="g1")
                nc.scalar.activation(
                    g1[:], ap_t[:], AFT.Gelu, bias=be_sb[:, a : a + 1], scale=1.0
                )
                nc.tensor.matmul(
                    sc_ps[:],
                    wf1_sb[:, a : a + 1],
                    g1[:],
                    start=(a == 0),
                    stop=(a == AA - 1),
                )

            # softmax over regions, per batch
            sc_sb = sm_pool.tile([1, ROWS], f32, tag="scsb")
            nc.vector.tensor_copy(sc_sb[:], sc_ps[:])
            for b in range(NB):
                gb = nb * NB + b
                nc.vector.tensor_scalar_add(
                    sc_sb[0:1, b * R : (b + 1) * R],
                    sc_sb[0:1, b * R : (b + 1) * R],
                    sc2_sb[0:1, gb : gb + 1],
                )
            sc2d = sm_pool.tile([NB, R], f32, tag="sc2d")
            nc.scalar.dma_start(sc2d[:], sc_sb[:])
            negmx = sm_pool.tile([NB, 1], f32, tag="negmx")
            nc.vector.reduce_max(
                negmx[:], sc2d[:], axis=mybir.AxisListType.X, negate=True
            )
            # exp via tanh so ScalarE stays on the gelu table set (a gelu<->exp
            # switch costs a 1.3us ACT_TABLE_LOAD twice per block):
            # exp(x) = (1+t)/(1-t), t = tanh(x/2), x = scores - max <= 0
            nmh = sm_pool.tile([NB, 1], f32, tag="nmh")
            nc.vector.tensor_scalar_mul(nmh[:], negmx[:], 0.5)
            tnh = sm_pool.tile([NB, R], f32, tag="tnh")
            nc.scalar.activation(tnh[:], sc2d[:], AFT.Tanh, bias=nmh[:], scale=0.5)
            den = sm_pool.tile([NB, R], f32, tag="den")
            nc.vector.tensor_scalar(
                den[:], tnh[:], -1.0, 1.0,
                op0=mybir.AluOpType.mult, op1=mybir.AluOpType.add,
            )
            rcpd = sm_pool.tile([NB, R], f32, tag="rcpd")
            nc.vector.reciprocal(rcpd[:], den[:])
            num = sm_pool.tile([NB, R], f32, tag="num")
            nc.vector.tensor_scalar_add(num[:], tnh[:], 1.0)
            esb = sm_pool.tile([NB, R], f32, tag="esb")
            nc.vector.tensor_mul(esb[:], num[:], rcpd[:])
            ssum = sm_pool.tile([NB, 1], f32, tag="ssum")
            nc.vector.reduce_sum(ssum[:], esb[:], axis=mybir.AxisListType.X)
            rcp = sm_pool.tile([NB, 1], f32, tag="rcp")
            nc.vector.reciprocal(rcp[:], ssum[:])
            alpha_sb = sm_pool.tile([NB, R], f32r, tag="alpha")
            nc.vector.tensor_scalar_mul(alpha_sb[:], esb[:], rcp[:])
            nc.scalar.dma_start(
                alpha_out[nb * NB : (nb + 1) * NB, :],
                alpha_sb[:].bitcast(f32),
            )

            # scatter alpha into block-diagonal stationary operand
            # bd_t[p, t, b] = alpha of block-row (t*128+p) if that row belongs
            # to batch b (rows [b*R, (b+1)*R) within the block), else 0.
            bd_t = sm_pool.tile([128, N_ROWT, NB], f32r, tag="bdt")
            nc.gpsimd.dma_start(bd_t[:], zrow[:])
            for b in range(NB):
                for t in range(N_ROWT):
                    rs = max(b * R, t * 128)
                    re_ = min((b + 1) * R, (t + 1) * 128)
                    if rs >= re_:
                        continue
                    nc.gpsimd.dma_start(
                        bd_t[rs - t * 128 : re_ - t * 128, t, b],
                        alpha_sb[b : b + 1, rs - b * R : re_ - b * R],
                    )

            # deferred stage 3 of the previous block
            if prev_bd is not None:
                stage3(nb - 1, prev_bd)
            prev_bd = bd_t

        stage3(N_BLOCKS - 1, prev_bd)

    nc.compile()
    return nc


def make_in_maps(inputs):
    """Split full inputs into per-core input maps (host-side prep)."""
    image = np.asarray(inputs["image"], dtype=np.float32)
    dh = np.asarray(inputs["decoder_hidden"], dtype=np.float32)
    We = np.asarray(inputs["We"], dtype=np.float32)
    be = np.asarray(inputs["be"], dtype=np.float32)
    Wd = np.asarray(inputs["Wd"], dtype=np.float32)
    bd = np.asarray(inputs["bd"], dtype=np.float32)
    Wf = np.asarray(inputs["Wf"], dtype=np.float32)
    bf = np.asarray(inputs["bf"], dtype=np.float32)

    bf16 = ml_dtypes.bfloat16
    wet = np.ascontiguousarray(We.T).astype(bf16)              # [C, A]
    wdt = np.ascontiguousarray(Wd.T).astype(bf16)              # [H, A]
    be8 = np.ascontiguousarray(be.reshape(AA, 128).T)          # [128, AA]
    bd8 = np.ascontiguousarray(bd.reshape(AA, 128).T)
    wf1 = np.ascontiguousarray(Wf[0, :A].reshape(AA, 128).T)
    wf2 = np.ascontiguousarray(Wf[0, A:].reshape(AA, 128).T)
    bf1 = bf.reshape(1, 1)

    image_bf = image.astype(bf16)  # host precast for the transposed loads

    in_maps = []
    for i in range(N_CORES):
        sl = slice(i * B_CORE, (i + 1) * B_CORE)
        imgbt = np.zeros((C, PAD_ROWS), dtype=bf16)
        imgbt[:, : B_CORE * R] = image_bf[sl].reshape(B_CORE * R, C).T
        in_maps.append(
            {
                "img": np.ascontiguousarray(
                    image[sl].reshape(B_CORE * R, C)
                ),
                "imgbt": imgbt,
                "dht": np.ascontiguousarray(dh[sl].T).astype(bf16),
                "wet": wet,
                "wdt": wdt,
                "be8": be8,
                "bd8": bd8,
                "wf1": wf1,
                "wf2": wf2,
                "bf1": bf1,
            }
        )
    return in_maps


_NC_CACHE = {}


def kernel(**inputs):
    if "nc" not in _NC_CACHE:
        _NC_CACHE["nc"] = build_nc()
    nc = _NC_CACHE["nc"]
    in_maps = make_in_maps(inputs)
    res = run_bass_kernel_spmd(nc, in_maps, core_ids=list(range(N_CORES)))
    kernel.last_result = res
    attended = np.concatenate([res.results[i]["att_out"] for i in range(N_CORES)], 0)
    alpha = np.concatenate([res.results[i]["alpha_out"] for i in range(N_CORES)], 0)
    return attended, alpha


kernel.last_result = None


# revision 41
# speedup vs baseline: 1.3279x; 1.0661x over previous
"""Trainium2 Bass kernel for the region-attention module.

Computation (per batch b):
    attn1 = image[b] @ We.T + be            # [R, A]
    attn2 = dh[b] @ Wd.T + bd               # [A]
    scores = gelu(attn1) @ Wf1 + gelu(attn2) @ Wf2 + bf   # [R]
    alpha = softmax(scores)                 # [R]
    attended = alpha @ image[b]             # [C]

Sharding: data-parallel over batch across 8 NeuronCores (16 batches/core),
weights replicated.  No cross-device communication.

Per-core kernel strategy (B_CORE=16 batches, processed in 8 blocks of 2):
  - image rows (b,r flattened) loaded once in fp32 (contiguous DMA), kept in
    SBUF for stage 3.
  - stage-1 matmul needs C on partitions; a host-precast, host-transposed
    bf16 copy of the image (an extra kernel input [C, rows]) is loaded with
    one ordinary 3D-AP DMA per block (on-device DMA-transposes act as global
    DMA-serialization barriers and are bf16-only anyway).  bf16 stage-1
    keeps end-to-end relative error ~1e-3 (fp32 accumulate in PSUM).
  - float32r (fp32 with 11-bit mantissa, full-rate on the PE) is used for the
    small score/attended matmuls; walrus requires every producer of an f32r
    matmul operand to be f32r-typed, so those tiles/DRAM tensors are declared
    float32r (same byte width/bits as fp32).
  - attn1T[a, rows] = WeT_chunk.T @ imT_chunk accumulated over 16 c-chunks.
  - gelu+bias on ScalarE straight out of PSUM; scores via a skinny
    [128,1].T @ [128,rows] float32r matmul accumulated over 8 a-chunks.
  - softmax on [2, 196] tiles (VectorE/ScalarE).
  - stage 3: attended = alpha_blockdiag.T @ image_natural as float32r
    matmuls over row chunks (alpha scattered into a [128, 4, 2] block-diag
    operand by tiny SBUF->SBUF DMAs).
"""

import os
import sys

for _p in ("/opt/trn_rl_repo", "/root/.axon_site/_ro/trn_rl_repo"):
    if os.path.isdir(_p) and _p not in sys.path:
        sys.path.append(_p)

from contextlib import ExitStack

import ml_dtypes
import numpy as np

import concourse.bass as bass
import concourse.tile as tile
from concourse import bacc, mybir
from concourse.bass_utils import run_bass_kernel_spmd

AFT = mybir.ActivationFunctionType
DT = mybir.dt

# Problem shapes (hardcoded per contest contract)
B, R, C = 128, 196, 2048
H, A = 1024, 1024
N_CORES = 8
B_CORE = B // N_CORES        # 16 batches per core
NB = 2                       # batches per block
ROWS = NB * R                # 392 rows per block
PAD_ROWS = 3200              # padded total rows of the transposed bf16 image
N_BLOCKS = B_CORE // NB      # 8
N_ROWT = (ROWS + 127) // 128  # 4 row chunks per block (128,128,128,8)
CA = C // 128                # 16 c-chunks
AA = A // 128                # 8 a-chunks
HA = H // 128                # 8 h-chunks
CB = C // 512                # 4 column blocks for stage 3


def _row_cnt(t):
    return min(128, ROWS - t * 128)


def build_nc():
    f32, bf16, f32r = DT.float32, DT.bfloat16, DT.float32r
    nc = bacc.Bacc(
        "TRN2", target_bir_lowering=False, debug=False, num_devices=N_CORES
    )

    img = nc.dram_tensor("img", [B_CORE * R, C], f32r, kind="ExternalInput")
    imgbt = nc.dram_tensor("imgbt", [C, PAD_ROWS], bf16, kind="ExternalInput")
    dht = nc.dram_tensor("dht", [H, B_CORE], bf16, kind="ExternalInput")
    wet = nc.dram_tensor("wet", [C, A], bf16, kind="ExternalInput")
    wdt = nc.dram_tensor("wdt", [H, A], bf16, kind="ExternalInput")
    be8 = nc.dram_tensor("be8", [128, AA], f32, kind="ExternalInput")
    bd8 = nc.dram_tensor("bd8", [128, AA], f32, kind="ExternalInput")
    wf1 = nc.dram_tensor("wf1", [128, AA], f32r, kind="ExternalInput")
    wf2 = nc.dram_tensor("wf2", [128, AA], f32r, kind="ExternalInput")
    bf1 = nc.dram_tensor("bf1", [1, 1], f32, kind="ExternalInput")

    att_out = nc.dram_tensor("att_out", [B_CORE, C], f32, kind="ExternalOutput")
    alpha_out = nc.dram_tensor("alpha_out", [B_CORE, R], f32, kind="ExternalOutput")

    with tile.TileContext(nc) as tc, ExitStack() as ctx:
        wet_pool = ctx.enter_context(tc.tile_pool(name="wet", bufs=1))
        const_pool = ctx.enter_context(tc.tile_pool(name="const", bufs=1))
        nat_pool = ctx.enter_context(tc.tile_pool(name="nat", bufs=6))
        imt_pool = ctx.enter_context(tc.tile_pool(name="imt", bufs=5))
        g1_pool = ctx.enter_context(tc.tile_pool(name="g1", bufs=8))
        sm_pool = ctx.enter_context(tc.tile_pool(name="sm", bufs=2))
        out_pool = ctx.enter_context(tc.tile_pool(name="outs", bufs=1))
        apsum = ctx.enter_context(tc.tile_pool(name="apsum", bufs=4, space="PSUM"))
        spsum = ctx.enter_context(tc.tile_pool(name="spsum", bufs=2, space="PSUM"))
        opsum = ctx.enter_context(tc.tile_pool(name="opsum", bufs=2, space="PSUM"))

        # ---- persistent constants
        be_sb = const_pool.tile([128, AA], f32, tag="be")
        nc.scalar.dma_start(be_sb[:], be8[:])
        bd_sb = const_pool.tile([128, AA], f32, tag="bd")
        nc.scalar.dma_start(bd_sb[:], bd8[:])
        # f32r-typed DMAs must use the SWDGE (gpsimd) path: HWDGE f32r
        # descriptors corrupt concurrent DMA-transpose traffic (HW bug found
        # empirically; see probe10 — sync f32r load scrambles xbar output).
        wf1_sb = const_pool.tile([128, AA], f32r, tag="wf1")
        nc.gpsimd.dma_start(wf1_sb[:], wf1[:])
        wf2_sb = const_pool.tile([128, AA], f32r, tag="wf2")
        nc.gpsimd.dma_start(wf2_sb[:], wf2[:])
        bf_sb = const_pool.tile([1, 1], f32, tag="bf")
        nc.scalar.dma_start(bf_sb[:], bf1[:])
        sc2_sb = const_pool.tile([1, B_CORE], f32, tag="sc2sb")
        # zero source for DMA-laundered f32r zero fills (memset can't target
        # f32r, and engine writers of f32r matmul operands must round)
        zrow = const_pool.tile([128, N_ROWT * NB], f32r, tag="zrow")
        nc.vector.memset(zrow[:].bitcast(f32), 0.0)

        # ---- phase 0: decoder branch -> sc2_sb[1, B_CORE].  Loads issue up
        # front; the PE work is emitted after block 0's stage-1 (see loop) so
        # it does not head-of-line-block the PE while its weights load.
        ph0 = ctx.enter_context(tc.tile_pool(name="ph0", bufs=1))
        ph0g = ctx.enter_context(tc.tile_pool(name="ph0g", bufs=4))
        wdt_sb = []
        for h in range(HA):
            t = ph0.tile([128, A], bf16, tag=f"wdt{h}")
            nc.scalar.dma_start(t[:], wdt[h * 128 : (h + 1) * 128, :])
            wdt_sb.append(t)
        dht_sb = []
        for h in range(HA):
            t = ph0.tile([128, B_CORE], bf16, tag=f"dht{h}")
            nc.scalar.dma_start(t[:], dht[h * 128 : (h + 1) * 128, :])
            dht_sb.append(t)

        def phase0_compute():
            sc2_ps = spsum.tile([1, B_CORE], f32, tag="sc")
            for a in range(AA):
                a2_ps = apsum.tile([128, B_CORE], f32, tag="ap")
                for h in range(HA):
                    nc.tensor.matmul(
                        a2_ps[:],
                        wdt_sb[h][:, a * 128 : (a + 1) * 128],
                        dht_sb[h][:],
                        start=(h == 0),
                        stop=(h == HA - 1),
                    )
                g2 = ph0g.tile([128, B_CORE], f32r, tag="g2")
                nc.scalar.activation(
                    g2[:], a2_ps[:], AFT.Gelu, bias=bd_sb[:, a : a + 1], scale=1.0
                )
                nc.tensor.matmul(
                    sc2_ps[:],
                    wf2_sb[:, a : a + 1],
                    g2[:],
                    start=(a == 0),
                    stop=(a == AA - 1),
                )
            nc.vector.tensor_scalar_add(sc2_sb[:], sc2_ps[:], bf_sb[0:1, 0:1])

        # encoder weights (loaded while phase 0 computes on the PE)
        wet_sb = []
        for c in range(CA):
            t = wet_pool.tile([128, A], bf16, tag=f"wet{c}")
            nc.gpsimd.dma_start(t[:], wet[c * 128 : (c + 1) * 128, :])
            wet_sb.append(t)

        def stage3(nb, bd_t):
            """attended(nb) = alpha_blockdiag.T @ image_natural; emitted one
            block late so a slow softmax never head-of-line-blocks the PE."""
            r0 = nb * ROWS
            nat = []
            for t in range(N_ROWT):
                rt = _row_cnt(t)
                ntile = nat_pool.tile([128, C], f32r, tag="nat")
                nc.gpsimd.dma_start(
                    ntile[0:rt, :], img[r0 + t * 128 : r0 + t * 128 + rt, :]
                )
                nat.append(ntile)
            att_sb = out_pool.tile([NB, C], f32, tag="attsb")
            for cb in range(CB):
                at_ps = opsum.tile([NB, 512], f32, tag="atps")
                for t in range(N_ROWT):
                    rt = _row_cnt(t)
                    nc.tensor.matmul(
                        at_ps[:],
                        bd_t[0:rt, t, :],
                        nat[t][0:rt, cb * 512 : (cb + 1) * 512],
                        start=(t == 0),
                        stop=(t == N_ROWT - 1),
                    )
                nc.vector.tensor_copy(att_sb[:, cb * 512 : (cb + 1) * 512], at_ps[:])
            nc.scalar.dma_start(att_out[nb * NB : (nb + 1) * NB, :], att_sb[:])

        # ---- main loop over blocks of NB batches
        prev_bd = None
        for nb in range(N_BLOCKS):
            r0 = nb * ROWS

            # transposed-layout bf16 load from the host-transposed copy:
            # imt_all[p, c, r] = imgbt[c*128+p, r0+r], one 3D-AP DMA
            imt_all = imt_pool.tile([128, CA, ROWS], bf16, tag="imt")
            nc.sync.dma_start(
                imt_all[:],
                imgbt[:, r0 : r0 + ROWS].rearrange("(c p) r -> p c r", p=128),
            )

            # stage 1: attn1T chunks + gelu + score accumulation
            sc_ps = spsum.tile([1, ROWS], f32, tag="sc")
            for a in range(AA):
                ap_t = apsum.tile([128, ROWS], f32, tag="ap")
                for c in range(CA):
                    nc.tensor.matmul(
                        ap_t[:],
                        wet_sb[c][:, a * 128 : (a + 1) * 128],
                        imt_all[:, c, 0:ROWS],
                        start=(c == 0),
                        stop=(c == CA - 1),
                    )
                g1 = g1_pool.tile([128, ROWS], f32r, tag="g1")
                nc.scalar.activation(
                    g1[:], ap_t[:], AFT.Gelu, bias=be_sb[:, a : a + 1], scale=1.0
                )
                nc.tensor.matmul(
                    sc_ps[:],
                    wf1_sb[:, a : a + 1],
                    g1[:],
                    start=(a == 0),
                    stop=(a == AA - 1),
                )

            if nb == 0:
                phase0_compute()

            # softmax over regions, per batch
            sc_sb = sm_pool.tile([1, ROWS], f32, tag="scsb")
            nc.vector.tensor_copy(sc_sb[:], sc_ps[:])
            for b in range(NB):
                gb = nb * NB + b
                nc.vector.tensor_scalar_add(
                    sc_sb[0:1, b * R : (b + 1) * R],
                    sc_sb[0:1, b * R : (b + 1) * R],
                    sc2_sb[0:1, gb : gb + 1],
                )
            sc2d = sm_pool.tile([NB, R], f32, tag="sc2d")
            nc.scalar.dma_start(sc2d[:], sc_sb[:])
            negmx = sm_pool.tile([NB, 1], f32, tag="negmx")
            nc.vector.reduce_max(
                negmx[:], sc2d[:], axis=mybir.AxisListType.X, negate=True
            )
            # exp via tanh so ScalarE stays on the gelu table set (a gelu<->exp
            # switch costs a 1.3us ACT_TABLE_LOAD twice per block):
            # exp(x) = (1+t)/(1-t), t = tanh(x/2), x = scores - max <= 0
            nmh = sm_pool.tile([NB, 1], f32, tag="nmh")
            nc.vector.tensor_scalar_mul(nmh[:], negmx[:], 0.5)
            tnh = sm_pool.tile([NB, R], f32, tag="tnh")
            nc.scalar.activation(tnh[:], sc2d[:], AFT.Tanh, bias=nmh[:], scale=0.5)
            den = sm_pool.tile([NB, R], f32, tag="den")
            nc.vector.tensor_scalar(
                den[:], tnh[:], -1.0, 1.0,
                op0=mybir.AluOpType.mult, op1=mybir.AluOpType.add,
            )
            rcpd = sm_pool.tile([NB, R], f32, tag="rcpd")
            nc.vector.reciprocal(rcpd[:], den[:])
            num = sm_pool.tile([NB, R], f32, tag="num")
            nc.vector.tensor_scalar_add(num[:], tnh[:], 1.0)
            esb = sm_pool.tile([NB, R], f32, tag="esb")
            nc.vector.tensor_mul(esb[:], num[:], rcpd[:])
            ssum = sm_pool.tile([NB, 1], f32, tag="ssum")
            nc.vector.reduce_sum(ssum[:], esb[:], axis=mybir.AxisListType.X)
            rcp = sm_pool.tile([NB, 1], f32, tag="rcp")
            nc.vector.reciprocal(rcp[:], ssum[:])
            alpha_sb = sm_pool.tile([NB, R], f32r, tag="alpha")
            nc.vector.tensor_scalar_mul(alpha_sb[:], esb[:], rcp[:])
            nc.scalar.dma_start(
                alpha_out[nb * NB : (nb + 1) * NB, :],
                alpha_sb[:].bitcast(f32),
            )

            # scatter alpha into block-diagonal stationary operand
            # bd_t[p, t, b] = alpha of block-row (t*128+p) if that row belongs
            # to batch b (rows [b*R, (b+1)*R) within the block), else 0.
            bd_t = sm_pool.tile([128, N_ROWT, NB], f32r, tag="bdt")
            nc.gpsimd.dma_start(bd_t[:], zrow[:])
            for b in range(NB):
                for t in range(N_ROWT):
                    rs = max(b * R, t * 128)
                    re_ = min((b + 1) * R, (t + 1) * 128)
                    if rs >= re_:
                        continue
                    nc.gpsimd.dma_start(
                        bd_t[rs - t * 128 : re_ - t * 128, t, b],
                        alpha_sb[b : b + 1, rs - b * R : re_ - b * R],
                    )

            # deferred stage 3 of the previous block
            if prev_bd is not None:
                stage3(nb - 1, prev_bd)
            prev_bd = bd_t

        stage3(N_BLOCKS - 1, prev_bd)

    nc.compile()
    return nc


def make_in_maps(inputs):
    """Split full inputs into per-core input maps (host-side prep)."""
    image = np.asarray(inputs["image"], dtype=np.float32)
    dh = np.asarray(inputs["decoder_hidden"], dtype=np.float32)
    We = np.asarray(inputs["We"], dtype=np.float32)
    be = np.asarray(inputs["be"], dtype=np.float32)
    Wd = np.asarray(inputs["Wd"], dtype=np.float32)
    bd = np.asarray(inputs["bd"], dtype=np.float32)
    Wf = np.asarray(inputs["Wf"], dtype=np.float32)
    bf = np.asarray(inputs["bf"], dtype=np.float32)

    bf16 = ml_dtypes.bfloat16
    wet = np.ascontiguousarray(We.T).astype(bf16)              # [C, A]
    wdt = np.ascontiguousarray(Wd.T).astype(bf16)              # [H, A]
    be8 = np.ascontiguousarray(be.reshape(AA, 128).T)          # [128, AA]
    bd8 = np.ascontiguousarray(bd.reshape(AA, 128).T)
    wf1 = np.ascontiguousarray(Wf[0, :A].reshape(AA, 128).T)
    wf2 = np.ascontiguousarray(Wf[0, A:].reshape(AA, 128).T)
    bf1 = bf.reshape(1, 1)

    image_bf = image.astype(bf16)  # host precast for the transposed loads

    in_maps = []
    for i in range(N_CORES):
        sl = slice(i * B_CORE, (i + 1) * B_CORE)
        imgbt = np.zeros((C, PAD_ROWS), dtype=bf16)
        imgbt[:, : B_CORE * R] = image_bf[sl].reshape(B_CORE * R, C).T
        in_maps.append(
            {
                "img": np.ascontiguousarray(
                    image[sl].reshape(B_CORE * R, C)
                ),
                "imgbt": imgbt,
                "dht": np.ascontiguousarray(dh[sl].T).astype(bf16),
                "wet": wet,
                "wdt": wdt,
                "be8": be8,
                "bd8": bd8,
                "wf1": wf1,
                "wf2": wf2,
                "bf1": bf1,
            }
        )
    return in_maps


_NC_CACHE = {}


def kernel(**inputs):
    if "nc" not in _NC_CACHE:
        _NC_CACHE["nc"] = build_nc()
    nc = _NC_CACHE["nc"]
    in_maps = make_in_maps(inputs)
    res = run_bass_kernel_spmd(nc, in_maps, core_ids=list(range(N_CORES)))
    kernel.last_result = res
    attended = np.concatenate([res.results[i]["att_out"] for i in range(N_CORES)], 0)
    alpha = np.concatenate([res.results[i]["alpha_out"] for i in range(N_CORES)], 0)
    return attended, alpha


kernel.last_result = None


# revision 42
# speedup vs baseline: 1.3424x; 1.0109x over previous
"""Trainium2 Bass kernel for the region-attention module.

Computation (per batch b):
    attn1 = image[b] @ We.T + be            # [R, A]
    attn2 = dh[b] @ Wd.T + bd               # [A]
    scores = gelu(attn1) @ Wf1 + gelu(attn2) @ Wf2 + bf   # [R]
    alpha = softmax(scores)                 # [R]
    attended = alpha @ image[b]             # [C]

Sharding: data-parallel over batch across 8 NeuronCores (16 batches/core),
weights replicated.  No cross-device communication.

Per-core kernel strategy (B_CORE=16 batches, processed in 8 blocks of 2):
  - image rows (b,r flattened) loaded once in fp32 (contiguous DMA), kept in
    SBUF for stage 3.
  - stage-1 matmul needs C on partitions; a host-precast, host-transposed
    bf16 copy of the image (an extra kernel input [C, rows]) is loaded with
    one ordinary 3D-AP DMA per block (on-device DMA-transposes act as global
    DMA-serialization barriers and are bf16-only anyway).  bf16 stage-1
    keeps end-to-end relative error ~1e-3 (fp32 accumulate in PSUM).
  - float32r (fp32 with 11-bit mantissa, full-rate on the PE) is used for the
    small score/attended matmuls; walrus requires every producer of an f32r
    matmul operand to be f32r-typed, so those tiles/DRAM tensors are declared
    float32r (same byte width/bits as fp32).
  - attn1T[a, rows] = WeT_chunk.T @ imT_chunk accumulated over 16 c-chunks.
  - gelu+bias on ScalarE straight out of PSUM; scores via a skinny
    [128,1].T @ [128,rows] float32r matmul accumulated over 8 a-chunks.
  - softmax on [2, 196] tiles (VectorE/ScalarE).
  - stage 3: attended = alpha_blockdiag.T @ image_natural as float32r
    matmuls over row chunks (alpha scattered into a [128, 4, 2] block-diag
    operand by tiny SBUF->SBUF DMAs).
"""

import os
import sys

for _p in ("/opt/trn_rl_repo", "/root/.axon_site/_ro/trn_rl_repo"):
    if os.path.isdir(_p) and _p not in sys.path:
        sys.path.append(_p)

from contextlib import ExitStack

import ml_dtypes
import numpy as np

import concourse.bass as bass
import concourse.tile as tile
from concourse import bacc, mybir
from concourse.bass_utils import run_bass_kernel_spmd

AFT = mybir.ActivationFunctionType
DT = mybir.dt

# Problem shapes (hardcoded per contest contract)
B, R, C = 128, 196, 2048
H, A = 1024, 1024
N_CORES = 8
B_CORE = B // N_CORES        # 16 batches per core
NB = 2                       # batches per block
ROWS = NB * R                # 392 rows per block
PAD_ROWS = 3200              # padded total rows of the transposed bf16 image
N_BLOCKS = B_CORE // NB      # 8
N_ROWT = (ROWS + 127) // 128  # 4 row chunks per block (128,128,128,8)
CA = C // 128                # 16 c-chunks
AA = A // 128                # 8 a-chunks
HA = H // 128                # 8 h-chunks
CB = C // 512                # 4 column blocks for stage 3


def _row_cnt(t):
    return min(128, ROWS - t * 128)


def build_nc():
    f32, bf16, f32r = DT.float32, DT.bfloat16, DT.float32r
    nc = bacc.Bacc(
        "TRN2", target_bir_lowering=False, debug=False, num_devices=N_CORES
    )

    img = nc.dram_tensor("img", [B_CORE * R, C], f32r, kind="ExternalInput")
    imgbt = nc.dram_tensor("imgbt", [C, PAD_ROWS], bf16, kind="ExternalInput")
    dht = nc.dram_tensor("dht", [H, B_CORE], bf16, kind="ExternalInput")
    wet = nc.dram_tensor("wet", [C, A], bf16, kind="ExternalInput")
    wdt = nc.dram_tensor("wdt", [H, A], bf16, kind="ExternalInput")
    be8 = nc.dram_tensor("be8", [128, AA], f32, kind="ExternalInput")
    bd8 = nc.dram_tensor("bd8", [128, AA], f32, kind="ExternalInput")
    wf1 = nc.dram_tensor("wf1", [128, AA], f32r, kind="ExternalInput")
    wf2 = nc.dram_tensor("wf2", [128, AA], f32r, kind="ExternalInput")
    bf1 = nc.dram_tensor("bf1", [1, 1], f32, kind="ExternalInput")

    att_out = nc.dram_tensor("att_out", [B_CORE, C], f32, kind="ExternalOutput")
    alpha_out = nc.dram_tensor("alpha_out", [B_CORE, R], f32, kind="ExternalOutput")

    with tile.TileContext(nc) as tc, ExitStack() as ctx:
        wet_pool = ctx.enter_context(tc.tile_pool(name="wet", bufs=1))
        const_pool = ctx.enter_context(tc.tile_pool(name="const", bufs=1))
        nat_pool = ctx.enter_context(tc.tile_pool(name="nat", bufs=6))
        imt_pool = ctx.enter_context(tc.tile_pool(name="imt", bufs=5))
        g1_pool = ctx.enter_context(tc.tile_pool(name="g1", bufs=8))
        sm_pool = ctx.enter_context(tc.tile_pool(name="sm", bufs=2))
        out_pool = ctx.enter_context(tc.tile_pool(name="outs", bufs=1))
        apsum = ctx.enter_context(tc.tile_pool(name="apsum", bufs=4, space="PSUM"))
        spsum = ctx.enter_context(tc.tile_pool(name="spsum", bufs=2, space="PSUM"))
        opsum = ctx.enter_context(tc.tile_pool(name="opsum", bufs=2, space="PSUM"))

        # ---- persistent constants
        be_sb = const_pool.tile([128, AA], f32, tag="be")
        nc.scalar.dma_start(be_sb[:], be8[:])
        bd_sb = const_pool.tile([128, AA], f32, tag="bd")
        nc.scalar.dma_start(bd_sb[:], bd8[:])
        # f32r-typed DMAs must use the SWDGE (gpsimd) path: HWDGE f32r
        # descriptors corrupt concurrent DMA-transpose traffic (HW bug found
        # empirically; see probe10 — sync f32r load scrambles xbar output).
        wf1_sb = const_pool.tile([128, AA], f32r, tag="wf1")
        nc.gpsimd.dma_start(wf1_sb[:], wf1[:])
        wf2_sb = const_pool.tile([128, AA], f32r, tag="wf2")
        nc.gpsimd.dma_start(wf2_sb[:], wf2[:])
        bf_sb = const_pool.tile([1, 1], f32, tag="bf")
        nc.scalar.dma_start(bf_sb[:], bf1[:])
        sc2_sb = const_pool.tile([1, B_CORE], f32, tag="sc2sb")
        # zero source for DMA-laundered f32r zero fills (memset can't target
        # f32r, and engine writers of f32r matmul operands must round)
        zrow = const_pool.tile([128, N_ROWT * NB], f32r, tag="zrow")
        nc.vector.memset(zrow[:].bitcast(f32), 0.0)

        # ---- phase 0: decoder branch -> sc2_sb[1, B_CORE].  Loads issue up
        # front; the PE work is emitted after block 0's stage-1 (see loop) so
        # it does not head-of-line-block the PE while its weights load.
        ph0 = ctx.enter_context(tc.tile_pool(name="ph0", bufs=1))
        ph0g = ctx.enter_context(tc.tile_pool(name="ph0g", bufs=4))
        wdt_sb = []
        for h in range(HA):
            t = ph0.tile([128, A], bf16, tag=f"wdt{h}")
            nc.scalar.dma_start(t[:], wdt[h * 128 : (h + 1) * 128, :])
            wdt_sb.append(t)
        dht_sb = []
        for h in range(HA):
            t = ph0.tile([128, B_CORE], bf16, tag=f"dht{h}")
            nc.scalar.dma_start(t[:], dht[h * 128 : (h + 1) * 128, :])
            dht_sb.append(t)

        def phase0_compute():
            sc2_ps = spsum.tile([1, B_CORE], f32, tag="sc")
            for a in range(AA):
                a2_ps = apsum.tile([128, B_CORE], f32, tag="ap")
                for h in range(HA):
                    nc.tensor.matmul(
                        a2_ps[:],
                        wdt_sb[h][:, a * 128 : (a + 1) * 128],
                        dht_sb[h][:],
                        start=(h == 0),
                        stop=(h == HA - 1),
                    )
                g2 = ph0g.tile([128, B_CORE], f32r, tag="g2")
                nc.scalar.activation(
                    g2[:], a2_ps[:], AFT.Gelu, bias=bd_sb[:, a : a + 1], scale=1.0
                )
                nc.tensor.matmul(
                    sc2_ps[:],
                    wf2_sb[:, a : a + 1],
                    g2[:],
                    start=(a == 0),
                    stop=(a == AA - 1),
                )
            nc.vector.tensor_scalar_add(sc2_sb[:], sc2_ps[:], bf_sb[0:1, 0:1])

        # encoder weights (loaded while phase 0 computes on the PE)
        wet_sb = []
        for c in range(CA):
            t = wet_pool.tile([128, A], bf16, tag=f"wet{c}")
            nc.gpsimd.dma_start(t[:], wet[c * 128 : (c + 1) * 128, :])
            wet_sb.append(t)

        def stage3(nb, bd_t):
            """attended(nb) = alpha_blockdiag.T @ image_natural; emitted one
            block late so a slow softmax never head-of-line-blocks the PE."""
            r0 = nb * ROWS
            nat = []
            for t in range(N_ROWT):
                rt = _row_cnt(t)
                ntile = nat_pool.tile([128, C], f32r, tag="nat")
                nc.gpsimd.dma_start(
                    ntile[0:rt, :], img[r0 + t * 128 : r0 + t * 128 + rt, :]
                )
                nat.append(ntile)
            att_sb = out_pool.tile([NB, C], f32, tag="attsb")
            for cb in range(CB):
                at_ps = opsum.tile([NB, 512], f32, tag="atps")
                for t in range(N_ROWT):
                    rt = _row_cnt(t)
                    nc.tensor.matmul(
                        at_ps[:],
                        bd_t[0:rt, t, :],
                        nat[t][0:rt, cb * 512 : (cb + 1) * 512],
                        start=(t == 0),
                        stop=(t == N_ROWT - 1),
                    )
                nc.vector.tensor_copy(att_sb[:, cb * 512 : (cb + 1) * 512], at_ps[:])
            nc.scalar.dma_start(att_out[nb * NB : (nb + 1) * NB, :], att_sb[:])

        # ---- main loop over blocks of NB batches
        prev_bd = None
        for nb in range(N_BLOCKS):
            r0 = nb * ROWS

            # transposed-layout bf16 load from the host-transposed copy:
            # imt_all[p, c, r] = imgbt[c*128+p, r0+r], one 3D-AP DMA
            imt_all = imt_pool.tile([128, CA, ROWS], bf16, tag="imt")
            nc.sync.dma_start(
                imt_all[:],
                imgbt[:, r0 : r0 + ROWS].rearrange("(c p) r -> p c r", p=128),
            )

            # stage 1: attn1T chunks + gelu + score accumulation
            sc_ps = spsum.tile([1, ROWS], f32, tag="sc")
            g1s = []
            for a in range(AA):
                ap_t = apsum.tile([128, ROWS], f32, tag="ap")
                for c in range(CA):
                    nc.tensor.matmul(
                        ap_t[:],
                        wet_sb[c][:, a * 128 : (a + 1) * 128],
                        imt_all[:, c, 0:ROWS],
                        start=(c == 0),
                        stop=(c == CA - 1),
                    )
                g1 = g1_pool.tile([128, ROWS], f32r, tag="g1")
                nc.scalar.activation(
                    g1[:], ap_t[:], AFT.Gelu, bias=be_sb[:, a : a + 1], scale=1.0
                )
                g1s.append(g1)
            # score matmuls clustered: one bf16<->f32r PE mode transition per
            # block instead of eight (transitions break FWL on following LDW)
            for a in range(AA):
                nc.tensor.matmul(
                    sc_ps[:],
                    wf1_sb[:, a : a + 1],
                    g1s[a][:],
                    start=(a == 0),
                    stop=(a == AA - 1),
                )

            if nb == 0:
                phase0_compute()

            # softmax over regions, per batch
            sc_sb = sm_pool.tile([1, ROWS], f32, tag="scsb")
            nc.vector.tensor_copy(sc_sb[:], sc_ps[:])
            for b in range(NB):
                gb = nb * NB + b
                nc.vector.tensor_scalar_add(
                    sc_sb[0:1, b * R : (b + 1) * R],
                    sc_sb[0:1, b * R : (b + 1) * R],
                    sc2_sb[0:1, gb : gb + 1],
                )
            sc2d = sm_pool.tile([NB, R], f32, tag="sc2d")
            nc.scalar.dma_start(sc2d[:], sc_sb[:])
            negmx = sm_pool.tile([NB, 1], f32, tag="negmx")
            nc.vector.reduce_max(
                negmx[:], sc2d[:], axis=mybir.AxisListType.X, negate=True
            )
            # exp via tanh so ScalarE stays on the gelu table set (a gelu<->exp
            # switch costs a 1.3us ACT_TABLE_LOAD twice per block):
            # exp(x) = (1+t)/(1-t), t = tanh(x/2), x = scores - max <= 0
            nmh = sm_pool.tile([NB, 1], f32, tag="nmh")
            nc.vector.tensor_scalar_mul(nmh[:], negmx[:], 0.5)
            tnh = sm_pool.tile([NB, R], f32, tag="tnh")
            nc.scalar.activation(tnh[:], sc2d[:], AFT.Tanh, bias=nmh[:], scale=0.5)
            den = sm_pool.tile([NB, R], f32, tag="den")
            nc.vector.tensor_scalar(
                den[:], tnh[:], -1.0, 1.0,
                op0=mybir.AluOpType.mult, op1=mybir.AluOpType.add,
            )
            rcpd = sm_pool.tile([NB, R], f32, tag="rcpd")
            nc.vector.reciprocal(rcpd[:], den[:])
            num = sm_pool.tile([NB, R], f32, tag="num")
            nc.vector.tensor_scalar_add(num[:], tnh[:], 1.0)
            esb = sm_pool.tile([NB, R], f32, tag="esb")
            nc.vector.tensor_mul(esb[:], num[:], rcpd[:])
            ssum = sm_pool.tile([NB, 1], f32, tag="ssum")
            nc.vector.reduce_sum(ssum[:], esb[:], axis=mybir.AxisListType.X)
            rcp = sm_pool.tile([NB, 1], f32, tag="rcp")
            nc.vector.reciprocal(rcp[:], ssum[:])
            alpha_sb = sm_pool.tile([NB, R], f32r, tag="alpha")
            nc.vector.tensor_scalar_mul(alpha_sb[:], esb[:], rcp[:])
            nc.scalar.dma_start(
                alpha_out[nb * NB : (nb + 1) * NB, :],
                alpha_sb[:].bitcast(f32),
            )

            # scatter alpha into block-diagonal stationary operand
            # bd_t[p, t, b] = alpha of block-row (t*128+p) if that row belongs
            # to batch b (rows [b*R, (b+1)*R) within the block), else 0.
            bd_t = sm_pool.tile([128, N_ROWT, NB], f32r, tag="bdt")
            nc.gpsimd.dma_start(bd_t[:], zrow[:])
            for b in range(NB):
                for t in range(N_ROWT):
                    rs = max(b * R, t * 128)
                    re_ = min((b + 1) * R, (t + 1) * 128)
                    if rs >= re_:
                        continue
                    nc.gpsimd.dma_start(
                        bd_t[rs - t * 128 : re_ - t * 128, t, b],
                        alpha_sb[b : b + 1, rs - b * R : re_ - b * R],
                    )

            # deferred stage 3 of the previous block
            if prev_bd is not None:
                stage3(nb - 1, prev_bd)
            prev_bd = bd_t

        stage3(N_BLOCKS - 1, prev_bd)

    nc.compile()
    return nc


def make_in_maps(inputs):
    """Split full inputs into per-core input maps (host-side prep)."""
    image = np.asarray(inputs["image"], dtype=np.float32)
    dh = np.asarray(inputs["decoder_hidden"], dtype=np.float32)
    We = np.asarray(inputs["We"], dtype=np.float32)
    be = np.asarray(inputs["be"], dtype=np.float32)
    Wd = np.asarray(inputs["Wd"], dtype=np.float32)
    bd = np.asarray(inputs["bd"], dtype=np.float32)
    Wf = np.asarray(inputs["Wf"], dtype=np.float32)
    bf = np.asarray(inputs["bf"], dtype=np.float32)

    bf16 = ml_dtypes.bfloat16
    wet = np.ascontiguousarray(We.T).astype(bf16)              # [C, A]
    wdt = np.ascontiguousarray(Wd.T).astype(bf16)              # [H, A]
    be8 = np.ascontiguousarray(be.reshape(AA, 128).T)          # [128, AA]
    bd8 = np.ascontiguousarray(bd.reshape(AA, 128).T)
    wf1 = np.ascontiguousarray(Wf[0, :A].reshape(AA, 128).T)
    wf2 = np.ascontiguousarray(Wf[0, A:].reshape(AA, 128).T)
    bf1 = bf.reshape(1, 1)

    image_bf = image.astype(bf16)  # host precast for the transposed loads

    in_maps = []
    for i in range(N_CORES):
        sl = slice(i * B_CORE, (i + 1) * B_CORE)
        imgbt = np.zeros((C, PAD_ROWS), dtype=bf16)
        imgbt[:, : B_CORE * R] = image_bf[sl].reshape(B_CORE * R, C).T
        in_maps.append(
            {
                "img": np.ascontiguousarray(
                    image[sl].reshape(B_CORE * R, C)
                ),
                "imgbt": imgbt,
                "dht": np.ascontiguousarray(dh[sl].T).astype(bf16),
                "wet": wet,
                "wdt": wdt,
                "be8": be8,
                "bd8": bd8,
                "wf1": wf1,
                "wf2": wf2,
                "bf1": bf1,
            }
        )
    return in_maps


_NC_CACHE = {}


def kernel(**inputs):
    if "nc" not in _NC_CACHE:
        _NC_CACHE["nc"] = build_nc()
    nc = _NC_CACHE["nc"]
    in_maps = make_in_maps(inputs)
    res = run_bass_kernel_spmd(nc, in_maps, core_ids=list(range(N_CORES)))
    kernel.last_result = res
    attended = np.concatenate([res.results[i]["att_out"] for i in range(N_CORES)], 0)
    alpha = np.concatenate([res.results[i]["alpha_out"] for i in range(N_CORES)], 0)
    return attended, alpha


kernel.last_result = None
